# revision 36
# baseline (speedup 1.0000x reference)
"""Multi-head attention forward, distributed over 8 TRN2 NeuronCores.

Sharding: sequence-parallel. Each core owns S/8 = 256 query rows per batch
(512 rows total, batch-major). It computes K^T and V' projections for its own
row shard, all-gathers K^T and V' across the 8 cores in four pipelined
2-head-pair chunks (each triggered as soon as its projections land), then
computes all 16 heads of attention for its query rows plus the output
projection — output rows are disjoint across cores, so there is no reduce at
the end.

Everything on-device stays in the "transposed" layout (feature dim on
partitions) so no transposes are ever needed:
  QT/KT: [d, s]  (d on partitions)    scores^T: [keys, queries]
  V':    [s, d]  (keys on partitions) attn_out^T: [d, queries]

V' ships through the all-gather already in its PV-ready interleaved layout
[key, (b, j, feat0..63, ones)] — the softmax-denominator ones column is baked
into the pack on the producer side, so the consumer needs exactly one
contiguous DMA per (dt, head-half) and the PV matmul (M=65) emits the
denominator on partition 64 for free.

Softmax: scores are bounded (|s| < 9 measured), so exp() without
max-subtraction is safe. exp is split across two engines per key-tile:
ACT runs the LUT exp; DVE runs a Schraudolph bit-trick exp directly in bf16
(one scalar_tensor_tensor emitting int16 bits: y = floor(x*128*log2e + B),
reinterpreted as bf16; rms rel err ~1.8% on the DVE-assigned tiles).

Output projection is split: head-pairs 0..6 are contracted into fp32 SBUF
partials while the tail of attention still runs; only head-pair 7's matmul,
one add, and the store remain after attention.

Compute dtype bf16 (fp32 PSUM accumulation).
"""

import sys

sys.path.insert(0, "/opt/trn_rl_repo")

import numpy as np
import ml_dtypes

import concourse.bass as bass
import concourse.mybir as mybir
import concourse.tile as tile
from concourse import bacc
from concourse.bass_utils import run_bass_kernel_spmd

R = 8          # cores
B = 2
S = 2048
D = 1024
H = 16
DK = 64
SQ = S // R    # 256 queries per batch per core
ROWS = B * SQ  # 512 rows per core, batch-major
CT = D // 128  # 8 contraction tiles
NKT = S // 128  # 16 key tiles per batch

BF16 = mybir.dt.bfloat16
F32 = mybir.dt.float32
I16 = mybir.dt.int16
EXP = mybir.ActivationFunctionType.Exp
COPY = mybir.ActivationFunctionType.Copy
MULT = mybir.AluOpType.mult
ADD = mybir.AluOpType.add
NP_BF16 = ml_dtypes.bfloat16

# Schraudolph bf16 exp: bits = floor(x * 128*log2e + SCH_B), viewed as bf16.
# DVE f32->int16 conversion truncates (measured), so SCH_B is calibrated for
# floor semantics (c = 6.5).
SCH_A = 128.0 * 1.4426950408889634
SCH_B = 127.0 * 128.0 - 6.5
# half-tiles (kt*2+hp) handled by the DVE exp (rest go to ACT); 16 of 32 —
# all odd halves, so every kt runs one ACT exp and one DVE exp concurrently
DVE_HALF = frozenset(range(1, 32, 2))

# all-gather chunking: equal chunks so the tail head-pairs' data arrives
# before the attention wavefront reaches them
CH_DT = [(0, 2), (2, 4), (4, 6), (6, 8)]
CH_OF = {dt: ch for ch, (s, e) in enumerate(CH_DT) for dt in range(s, e)}
NCH = len(CH_DT)
KT_DT = 128 * 512            # KT pack elements per dt
V_DT = 128 * 520             # V' pack elements per dt (2 hp * 4 slots * 65)
PACK_DT = KT_DT + V_DT


def build_graph(debug=False):
    nc = bacc.Bacc(None, target_bir_lowering=False, num_devices=R)

    # inputs arrive pre-arranged on the host to the exact SBUF layouts
    # ([p, ct, ...] with p the partition), so every load is contiguous
    xT = nc.declare_dram_parameter("xT", [128, CT * ROWS], BF16, isOutput=False)
    # wq/wk/wv are dt-major ([p, dt, ct, 128]) so chunk 0's slices load first
    wq = nc.declare_dram_parameter("wq", [128, CT * D], BF16, isOutput=False)
    wk = nc.declare_dram_parameter("wk", [128, CT * D], BF16, isOutput=False)
    wv = nc.declare_dram_parameter("wv", [128, CT * D], BF16, isOutput=False)
    wo = nc.declare_dram_parameter("wo", [128, CT * D], BF16, isOutput=False)
    out = nc.declare_dram_parameter("out", [ROWS, D], F32, isOutput=True)

    # Per-chunk packed bounce buffers.
    # KT region (per dt): flat d_local*512 + s with d_local = p.
    # V' region (per dt): flat p*520 + hp*260 + b*130 + j*65 + c, where the
    # batch-b key is k = r*256 + j*128 + p, feature d = dt*128 + hp*64 + c for
    # c in [0,64), and c = 64 is the constant-ones softmax column.
    cc_in_pack = [
        nc.dram_tensor(f"cc_in_pack{h}", [(e - s) * PACK_DT // 256, 256], BF16)
        for h, (s, e) in enumerate(CH_DT)
    ]
    cc_out_pack = [
        nc.dram_tensor(
            f"cc_out_pack{h}", [R * (e - s) * PACK_DT // 256, 256], BF16,
            addr_space="Shared",
        )
        for h, (s, e) in enumerate(CH_DT)
    ]
    groups = [list(range(R))]

    dbg = {}
    if debug:
        dbg["qt"] = nc.declare_dram_parameter("dbg_qt", [128, CT * ROWS], BF16, isOutput=True)
        dbg["at"] = nc.declare_dram_parameter("dbg_at", [128, CT * ROWS], BF16, isOutput=True)
        dbg["kt"] = nc.declare_dram_parameter("dbg_kt", [128, R * ROWS], BF16, isOutput=True)
        dbg["ve"] = nc.declare_dram_parameter("dbg_ve", [128, R * B * 2 * 65], BF16, isOutput=True)
        dbg["vo"] = nc.declare_dram_parameter("dbg_vo", [128, R * B * 2 * 65], BF16, isOutput=True)

    def pack_ap(tensor_ap, offset, dims):
        return bass.AP(tensor_ap.tensor, offset, dims)

    with tile.TileContext(nc) as tc:
        with tc.tile_pool(name="persist", bufs=1) as pp:
            xT_sb = pp.tile([128, CT, ROWS], BF16)
            wq_sb = pp.tile([128, CT, CT, 128], BF16)
            wkc = [pp.tile([128, e - s, CT, 128], BF16, name=f"wkc{h}")
                   for h, (s, e) in enumerate(CH_DT)]
            wvc = [pp.tile([128, e - s, CT, 128], BF16, name=f"wvc{h}")
                   for h, (s, e) in enumerate(CH_DT)]
            wo_sb = pp.tile([128, CT, D], BF16)
            qt_sb = pp.tile([128, CT, ROWS], BF16)
            at_sb = pp.tile([128, CT, ROWS], BF16)
            # double-buffered attention inputs, one buffer pair per dt parity
            kt2 = [pp.tile([128, R, ROWS], BF16, name=f"kt2_{i}") for i in range(2)]
            # V' per (r, b, j) slot: [data(64) | ones(1)]; ones arrive via AG
            v2e = [pp.tile([128, R, B, 2, 65], BF16, name=f"v2e_{i}") for i in range(2)]
            v2o = [pp.tile([128, R, B, 2, 65], BF16, name=f"v2o_{i}") for i in range(2)]
            ones_sb = pp.tile([128, 64], BF16)
            # V' pack staging, one per dt, ones columns memset once
            sbv = [pp.tile([128, 2, 4, 65], BF16, name=f"sbv_{d}") for d in range(CT)]
            # fp32 partials of the output projection (pass A: dt 0..5)
            oA = [pp.tile([128, 512], F32, name=f"oA_{t}") for t in range(8)]
            nc.vector.memset(ones_sb[:], 1.0)
            for d in range(CT):
                nc.vector.memset(sbv[d][:, :, :, 64:65], 1.0)

            # priority-ordered input loads, all on one ring so chunk 0's
            # K/V weights get full HBM bandwidth first
            def load_w(h):
                s, e = CH_DT[h]
                nc.sync.dma_start(
                    wkc[h][:], bass.AP(wk.ap().tensor, s * 1024,
                                       [[CT * D, 128], [1, (e - s) * 1024]]))
                nc.sync.dma_start(
                    wvc[h][:], bass.AP(wv.ap().tensor, s * 1024,
                                       [[CT * D, 128], [1, (e - s) * 1024]]))

            nc.sync.dma_start(xT_sb[:], xT.ap())
            load_w(0)
            nc.scalar.dma_start(wq_sb[:], wq.ap())
            nc.scalar.dma_start(wo_sb[:], wo.ap())

            # ---- stage A: K^T and V' projections + pipelined all-gathers ----
            with (
                tc.tile_pool(name="proj_ps", bufs=2, space="PSUM") as proj_ps,
                tc.tile_pool(name="stage", bufs=3) as stage,
            ):
                for ch, (dt_s, dt_e) in enumerate(CH_DT):
                    if ch + 1 < NCH:
                        load_w(ch + 1)
                    ndt = dt_e - dt_s
                    pk_in = cc_in_pack[ch].ap()
                    v_base = ndt * KT_DT
                    # K^T for this chunk's dts
                    for dt in range(dt_s, dt_e):
                        ps = proj_ps.tile([128, ROWS], F32, tag="ps")
                        for ct in range(CT):
                            nc.tensor.matmul(
                                ps[:],
                                wkc[ch][:, dt - dt_s, ct, :],
                                xT_sb[:, ct, :],
                                start=(ct == 0),
                                stop=(ct == CT - 1),
                            )
                        sb = stage.tile([128, ROWS], BF16, tag="kv")
                        nc.scalar.activation(sb[:], ps[:], COPY)
                        nc.sync.dma_start(
                            pack_ap(pk_in, (dt - dt_s) * KT_DT,
                                    [[512, 128], [1, 512]]),
                            sb[:],
                        )
                    # V' for this chunk's dts: per row-block st=(b,j), 2dt of
                    # features at once (N=256), copied into the interleaved
                    # staging tiles feature-slice by feature-slice
                    for st in range(ROWS // 128):
                        b_, j_ = st // 2, st % 2
                        ps = proj_ps.tile([128, ndt * 128], F32, tag="ps")
                        for ct in range(CT):
                            nc.tensor.matmul(
                                ps[:],
                                xT_sb[:, ct, st * 128 : (st + 1) * 128],
                                wvc[ch][:, :, ct, :],
                                start=(ct == 0),
                                stop=(ct == CT - 1),
                            )
                        for dtl in range(ndt):
                            # [128, hp:2, c:64] -> sbv[dt][:, hp, b*2+j, 0:64]
                            nc.vector.tensor_copy(
                                sbv[dt_s + dtl][:, :, b_ * 2 + j_, 0:64],
                                ps[:, dtl * 128 : (dtl + 1) * 128].rearrange(
                                    "p (hp c) -> p hp c", hp=2
                                ),
                            )
                    for dtl in range(ndt):
                        nc.sync.dma_start(
                            pack_ap(pk_in, v_base + dtl * V_DT,
                                    [[520, 128], [1, 520]]),
                            sbv[dt_s + dtl][:],
                        )
                    nc.gpsimd.collective_compute(
                        "AllGather",
                        mybir.AluOpType.bypass,
                        replica_groups=groups,
                        ins=[cc_in_pack[ch].ap().opt()],
                        outs=[cc_out_pack[ch].ap().opt()],
                    )

                # ---- stage B: Q^T projection (overlaps the collectives) ----
                for dt in range(CT):
                    ps = proj_ps.tile([128, ROWS], F32, tag="ps")
                    for ct in range(CT):
                        nc.tensor.matmul(
                            ps[:],
                            wq_sb[:, dt, ct, :],
                            xT_sb[:, ct, :],
                            start=(ct == 0),
                            stop=(ct == CT - 1),
                        )
                    nc.scalar.activation(qt_sb[:, dt, :], ps[:], COPY)

            # gathered pack reads (rank r block at r*PACK_ch)
            def kt_src(ch, ddl):
                ndt = CH_DT[ch][1] - CH_DT[ch][0]
                return bass.AP(
                    cc_out_pack[ch].ap().tensor,
                    ddl * KT_DT,
                    [[512, 128], [ndt * PACK_DT, R], [1, 512]],
                )

            def v_src(ch, ddl, hp):
                ndt = CH_DT[ch][1] - CH_DT[ch][0]
                return bass.AP(
                    cc_out_pack[ch].ap().tensor,
                    ndt * KT_DT + ddl * V_DT + hp * 260,
                    [[520, 128], [ndt * PACK_DT, R], [1, 260]],
                )

            def issue_loads(dt):
                # rank-halved loads: the kt loop consumes keys r-major, so
                # scoring starts as soon as ranks 0-3 land even while the
                # second half is still contending with collective HBM traffic
                ch = CH_OF[dt]
                ddl = dt - CH_DT[ch][0]
                ndt = CH_DT[ch][1] - CH_DT[ch][0]
                ks = kt_src(ch, ddl)
                ve_ap = v2e[dt % 2][:].rearrange("p r b j c -> p r (b j c)")
                vo_ap = v2o[dt % 2][:].rearrange("p r b j c -> p r (b j c)")
                vs0 = v_src(ch, ddl, 0)
                vs1 = v_src(ch, ddl, 1)
                for h in range(2):
                    rs = slice(h * 4, (h + 1) * 4)
                    roff = h * 4 * ndt * PACK_DT
                    nc.gpsimd.dma_start(
                        kt2[dt % 2][:, rs, :],
                        bass.AP(ks.tensor, ks.offset + roff,
                                [ks.ap[0], [ks.ap[1][0], 4], ks.ap[2]]),
                    )
                    nc.gpsimd.dma_start(
                        ve_ap[:, rs, :],
                        bass.AP(vs0.tensor, vs0.offset + roff,
                                [vs0.ap[0], [vs0.ap[1][0], 4], vs0.ap[2]]),
                    )
                    nc.gpsimd.dma_start(
                        vo_ap[:, rs, :],
                        bass.AP(vs1.tensor, vs1.offset + roff,
                                [vs1.ap[0], [vs1.ap[1][0], 4], vs1.ap[2]]),
                    )

            # ---- attention: 8 groups of (2 heads x 2 batches) ----
            issue_loads(0)
            issue_loads(1)
            with (
                tc.tile_pool(name="att_ps", bufs=2, space="PSUM") as att_psp,
                tc.tile_pool(name="pt", bufs=10) as ptp,
                tc.tile_pool(name="rec", bufs=4) as recp,
            ):
                for dt in range(CT):
                    k2 = kt2[dt % 2]
                    ve = v2e[dt % 2]
                    vo = v2o[dt % 2]
                    # one accumulator tile per (b, hp): separate tiles so each
                    # accumulation chain owns its PSUM bank (start=True clears
                    # has_written at bank granularity — chains must not share)
                    at_ps = [
                        [att_psp.tile([128, SQ], F32, tag="at", bufs=4,
                                      name=f"at_{dt}_{b}_{hp}")
                         for hp in range(2)]
                        for b in range(B)
                    ]
                    # software-pipelined kt loop: the PE queue is in-order, so
                    # PV(kt) issued right after scores(kt) head-blocks the
                    # queue on exp(kt). Delay each kt's PV matmuls until after
                    # the NEXT kt's scores have been issued — the PE streams
                    # scores(kt+1) while the exps of kt run.
                    def issue_pv(kt, pts):
                        rr, jh = kt // 2, kt % 2
                        for hp in range(2):
                            vt = ve if hp == 0 else vo
                            for b in range(B):
                                nc.tensor.matmul(
                                    at_ps[b][hp][0:65, :],
                                    vt[:, rr, b, jh, 0:65],
                                    pts[hp][:, b * SQ : (b + 1) * SQ],
                                    start=(kt == 0),
                                    stop=(kt == NKT - 1),
                                )

                    pipe = []
                    for kt in range(NKT):
                        rr, jh = kt // 2, kt % 2
                        pts = []
                        for hp in range(2):
                            hs = slice(hp * 64, (hp + 1) * 64)
                            st2 = att_psp.tile([128, 2 * SQ], F32, tag="st", bufs=4)
                            for b in range(B):
                                nc.tensor.matmul(
                                    st2[:, b * SQ : (b + 1) * SQ],
                                    k2[hs, rr, b * SQ + jh * 128 : b * SQ + jh * 128 + 128],
                                    qt_sb[hs, dt, b * SQ : (b + 1) * SQ],
                                    start=True,
                                    stop=True,
                                )
                            pt2 = ptp.tile([128, 2 * SQ], BF16, tag="pt")
                            if (kt * 2 + hp) % 32 in DVE_HALF:
                                nc.vector.tensor_scalar(
                                    pt2[:].bitcast(I16),
                                    st2[:],
                                    SCH_A,
                                    SCH_B,
                                    MULT,
                                    ADD,
                                )
                            else:
                                nc.scalar.activation(pt2[:], st2[:], EXP)
                            pts.append(pt2)
                        pipe.append(pts)
                        if kt >= 3:
                            # PV lags THREE kts behind scores: even when the
                            # exp engines fall behind during the normalize
                            # boundary, the in-order PE queue never waits
                            issue_pv(kt - 3, pipe[kt - 3])
                    for k in (NKT - 3, NKT - 2, NKT - 1):
                        issue_pv(k, pipe[k])
                    # prefetch dt+2's K^T/V' — issued AFTER this dt's last
                    # reads of the shared (dt%2)-parity buffers so the tile
                    # scheduler sees it as WAR (write waits for our reads),
                    # not RAW; it executes during dt+1's compute
                    if dt + 2 < CT:
                        issue_loads(dt + 2)
                    # normalize by the softmax sums (partition 64 of each
                    # accumulator): cast sums to bf16, broadcast across 64
                    # partitions with a 1-row ones-matmul on PE, reciprocal,
                    # multiply.
                    for b in range(B):
                        bcol = b * SQ
                        for hp in range(2):
                            ps = at_ps[b][hp]
                            sums = recp.tile([128, SQ], BF16, tag="sums")
                            bc_ps = att_psp.tile([64, SQ], F32, tag="st", bufs=4,
                                                 name=f"bc_{dt}_{b}_{hp}")
                            bc_sb = recp.tile([64, SQ], F32, tag="bcs")
                            nc.scalar.activation(sums[64:65, :], ps[64:65, :], COPY)
                            nc.tensor.matmul(
                                bc_ps[:],
                                ones_sb[64:65, :],
                                sums[64:65, :],
                                start=True,
                                stop=True,
                            )
                            nc.vector.reciprocal_approx_fast(bc_sb[:], bc_ps[:])
                            if hp == 0:
                                nc.vector.tensor_mul(
                                    at_sb[0:64, dt, bcol : bcol + SQ],
                                    ps[0:64, :],
                                    bc_sb[:],
                                )
                            else:
                                shift = recp.tile([64, SQ], BF16, tag="shift")
                                nc.vector.tensor_mul(shift[:], ps[0:64, :], bc_sb[:])
                                nc.sync.dma_start(
                                    at_sb[64:128, dt, bcol : bcol + SQ], shift[:]
                                )
                    if dt == 5:
                        # ---- output projection pass A: contract dt 0..5 of
                        # at_sb into fp32 SBUF partials inside the window
                        # where dt6/dt7 usually wait on the final all-gather
                        # chunk; borrows the idle "st" PSUM slots
                        for t in range(8):
                            st_, nh = t // 2, t % 2
                            ps = att_psp.tile([128, 512], F32, tag="st", bufs=4,
                                              name=f"oA_ps_{t}")
                            for d in range(6):
                                nc.tensor.matmul(
                                    ps[:],
                                    at_sb[:, d, st_ * 128 : (st_ + 1) * 128],
                                    wo_sb[:, d, nh * 512 : (nh + 1) * 512],
                                    start=(d == 0),
                                    stop=(d == 5),
                                )
                            nc.scalar.activation(oA[t][:], ps[:], COPY)

            if debug:
                nc.sync.dma_start(dbg["qt"].ap(), qt_sb[:])
                nc.sync.dma_start(dbg["at"].ap(), at_sb[:])
                nc.sync.dma_start(dbg["kt"].ap(), kt2[0][:])
                nc.sync.dma_start(
                    dbg["ve"].ap(), v2e[0][:].rearrange("p r b j c -> p (r b j c)")
                )
                nc.sync.dma_start(
                    dbg["vo"].ap(), v2o[0][:].rearrange("p r b j c -> p (r b j c)")
                )

            # ---- output projection pass B: last two head-pairs + add + store
            with (
                tc.tile_pool(name="oB_ps", bufs=3, space="PSUM") as obp,
                tc.tile_pool(name="oB_sb", bufs=3) as obs,
            ):
                for t in range(8):
                    st_, nh = t // 2, t % 2
                    ps = obp.tile([128, 512], F32, tag="oB")
                    for d in (6, 7):
                        nc.tensor.matmul(
                            ps[:],
                            at_sb[:, d, st_ * 128 : (st_ + 1) * 128],
                            wo_sb[:, d, nh * 512 : (nh + 1) * 512],
                            start=(d == 6),
                            stop=(d == 7),
                        )
                    osb = obs.tile([128, 512], F32, tag="os")
                    nc.vector.tensor_add(osb[:], ps[:], oA[t][:])
                    nc.sync.dma_start(
                        out[st_ * 128 : (st_ + 1) * 128, nh * 512 : (nh + 1) * 512],
                        osb[:],
                    )

    nc.compile()
    return nc


_NC = None


def _get_nc():
    global _NC
    if _NC is None:
        _NC = build_graph()
    return _NC


def _warr(w):
    # [d_in, d_out] -> [p, ct, d_out] flattened to [128, CT*D] (contiguous load)
    return np.ascontiguousarray(
        np.asarray(w, np.float32).reshape(CT, 128, D).transpose(1, 0, 2)
    ).astype(NP_BF16).reshape(128, CT * D)


def _warr_dt(w):
    # [d_in, d_out] -> [p, dt, ct, c] flattened (dt-major: chunk 0 loads first)
    return np.ascontiguousarray(
        np.asarray(w, np.float32).reshape(CT, 128, CT, 128).transpose(1, 2, 0, 3)
    ).astype(NP_BF16).reshape(128, CT * D)


def make_in_maps(x, W_q, W_k, W_v, W_o):
    wq = _warr_dt(np.asarray(W_q, np.float32) * 0.125)
    wk = _warr_dt(W_k)
    wv = _warr_dt(W_v)
    wo = _warr(W_o)
    x = np.asarray(x, np.float32)
    in_maps = []
    for r in range(R):
        shard = x[:, r * SQ : (r + 1) * SQ, :].reshape(ROWS, D)  # batch-major rows
        xT_r = np.ascontiguousarray(
            shard.T.reshape(CT, 128, ROWS).transpose(1, 0, 2)
        ).astype(NP_BF16).reshape(128, CT * ROWS)
        in_maps.append({"xT": xT_r, "wq": wq, "wk": wk, "wv": wv, "wo": wo})
    return in_maps


def assemble_out(results):
    full = np.zeros((B, S, D), np.float32)
    for r in range(R):
        o = np.asarray(results[r]["out"], np.float32)
        for b in range(B):
            full[b, r * SQ : (r + 1) * SQ, :] = o[b * SQ : (b + 1) * SQ, :]
    return full


def run(x, W_q, W_k, W_v, W_o, trace=False):
    nc = _get_nc()
    in_maps = make_in_maps(x, W_q, W_k, W_v, W_o)
    res = run_bass_kernel_spmd(nc, in_maps, core_ids=list(range(R)), trace=trace)
    return assemble_out(res.results), res


def kernel(x, W_q, W_k, W_v, W_o):
    out, _ = run(x, W_q, W_k, W_v, W_o)
    return out


# revision 37
# speedup vs baseline: 1.3405x; 1.3405x over previous
"""Multi-head attention forward, distributed over 8 TRN2 NeuronCores.

Sharding: sequence-parallel. Each core owns S/8 = 256 query rows per batch
(512 rows total, batch-major). It computes K^T and V' projections for its own
row shard, all-gathers K^T and V' across the 8 cores in four pipelined
2-head-pair chunks (each triggered as soon as its projections land), then
computes all 16 heads of attention for its query rows plus the output
projection — output rows are disjoint across cores, so there is no reduce at
the end.

Everything on-device stays in the "transposed" layout (feature dim on
partitions) so no transposes are ever needed:
  QT/KT: [d, s]  (d on partitions)    scores^T: [keys, queries]
  V':    [s, d]  (keys on partitions) attn_out^T: [d, queries]

V' ships through the all-gather already in its PV-ready interleaved layout
[key, (b, j, feat0..63, ones)] — the softmax-denominator ones column is baked
into the pack on the producer side, so the consumer needs exactly one
contiguous DMA per (dt, head-half) and the PV matmul (M=65) emits the
denominator on partition 64 for free.

Softmax: scores are bounded (|s| < 9 measured), so exp() without
max-subtraction is safe. exp is split across two engines per key-tile:
ACT runs the LUT exp; DVE runs a Schraudolph bit-trick exp directly in bf16
(one scalar_tensor_tensor emitting int16 bits: y = floor(x*128*log2e + B),
reinterpreted as bf16; rms rel err ~1.8% on the DVE-assigned tiles).

Output projection is split: head-pairs 0..6 are contracted into fp32 SBUF
partials while the tail of attention still runs; only head-pair 7's matmul,
one add, and the store remain after attention.

Compute dtype bf16 (fp32 PSUM accumulation).
"""

import sys

sys.path.insert(0, "/opt/trn_rl_repo")

import numpy as np
import ml_dtypes

import concourse.bass as bass
import concourse.mybir as mybir
import concourse.tile as tile
from concourse import bacc
from concourse.bass_utils import run_bass_kernel_spmd

R = 8          # cores
B = 2
S = 2048
D = 1024
H = 16
DK = 64
SQ = S // R    # 256 queries per batch per core
ROWS = B * SQ  # 512 rows per core, batch-major
CT = D // 128  # 8 contraction tiles
NKT = S // 128  # 16 key tiles per batch

BF16 = mybir.dt.bfloat16
F32 = mybir.dt.float32
I16 = mybir.dt.int16
EXP = mybir.ActivationFunctionType.Exp
COPY = mybir.ActivationFunctionType.Copy
MULT = mybir.AluOpType.mult
ADD = mybir.AluOpType.add
NP_BF16 = ml_dtypes.bfloat16

# Schraudolph bf16 exp: bits = floor(x * 128*log2e + SCH_B), viewed as bf16.
# DVE f32->int16 conversion truncates (measured), so SCH_B is calibrated for
# floor semantics (c = 6.5).
SCH_A = 128.0 * 1.4426950408889634
SCH_B = 127.0 * 128.0 - 6.5
# half-tiles (kt*2+hp) handled by the DVE exp (rest go to ACT); 16 of 32 —
# all odd halves, so every kt runs one ACT exp and one DVE exp concurrently
DVE_HALF = frozenset(range(1, 32, 2))

# all-gather chunking: equal chunks so the tail head-pairs' data arrives
# before the attention wavefront reaches them
CH_DT = [(0, 2), (2, 4), (4, 6), (6, 8)]
CH_OF = {dt: ch for ch, (s, e) in enumerate(CH_DT) for dt in range(s, e)}
NCH = len(CH_DT)
KT_DT = 128 * 512            # KT pack elements per dt
V_DT = 128 * 520             # V' pack elements per dt (2 hp * 4 slots * 65)
PACK_DT = KT_DT + V_DT


def build_graph(debug=False):
    nc = bacc.Bacc(None, target_bir_lowering=False, num_devices=R)

    # inputs arrive pre-arranged on the host to the exact SBUF layouts
    # ([p, ct, ...] with p the partition), so every load is contiguous
    xT = nc.declare_dram_parameter("xT", [128, CT * ROWS], BF16, isOutput=False)
    # wq/wk/wv are dt-major ([p, dt, ct, 128]) so chunk 0's slices load first
    wq = nc.declare_dram_parameter("wq", [128, CT * D], BF16, isOutput=False)
    wk = nc.declare_dram_parameter("wk", [128, CT * D], BF16, isOutput=False)
    wv = nc.declare_dram_parameter("wv", [128, CT * D], BF16, isOutput=False)
    wo = nc.declare_dram_parameter("wo", [128, CT * D], BF16, isOutput=False)
    out = nc.declare_dram_parameter("out", [ROWS, D], F32, isOutput=True)

    # Per-chunk packed bounce buffers.
    # KT region (per dt): flat d_local*512 + s with d_local = p.
    # V' region (per dt): flat p*520 + hp*260 + b*130 + j*65 + c, where the
    # batch-b key is k = r*256 + j*128 + p, feature d = dt*128 + hp*64 + c for
    # c in [0,64), and c = 64 is the constant-ones softmax column.
    cc_in_pack = [
        nc.dram_tensor(f"cc_in_pack{h}", [(e - s) * PACK_DT // 256, 256], BF16)
        for h, (s, e) in enumerate(CH_DT)
    ]
    cc_out_pack = [
        nc.dram_tensor(
            f"cc_out_pack{h}", [R * (e - s) * PACK_DT // 256, 256], BF16,
            addr_space="Shared",
        )
        for h, (s, e) in enumerate(CH_DT)
    ]
    groups = [list(range(R))]

    dbg = {}
    if debug:
        dbg["qt"] = nc.declare_dram_parameter("dbg_qt", [128, CT * ROWS], BF16, isOutput=True)
        dbg["at"] = nc.declare_dram_parameter("dbg_at", [128, CT * ROWS], BF16, isOutput=True)
        dbg["kt"] = nc.declare_dram_parameter("dbg_kt", [128, R * ROWS], BF16, isOutput=True)
        dbg["ve"] = nc.declare_dram_parameter("dbg_ve", [128, R * B * 2 * 65], BF16, isOutput=True)
        dbg["vo"] = nc.declare_dram_parameter("dbg_vo", [128, R * B * 2 * 65], BF16, isOutput=True)

    def pack_ap(tensor_ap, offset, dims):
        return bass.AP(tensor_ap.tensor, offset, dims)

    with tile.TileContext(nc) as tc:
        with tc.tile_pool(name="persist", bufs=1) as pp:
            xT_sb = pp.tile([128, CT, ROWS], BF16)
            wq_sb = pp.tile([128, CT, CT, 128], BF16)
            wkc = [pp.tile([128, e - s, CT, 128], BF16, name=f"wkc{h}")
                   for h, (s, e) in enumerate(CH_DT)]
            wvc = [pp.tile([128, e - s, CT, 128], BF16, name=f"wvc{h}")
                   for h, (s, e) in enumerate(CH_DT)]
            wo_sb = pp.tile([128, CT, D], BF16)
            qt_sb = pp.tile([128, CT, ROWS], BF16)
            at_sb = pp.tile([128, CT, ROWS], BF16)
            # double-buffered attention inputs, one buffer pair per dt parity
            kt2 = [pp.tile([128, R, ROWS], BF16, name=f"kt2_{i}") for i in range(2)]
            # V' per (r, b, j) slot: [data(64) | ones(1)]; ones arrive via AG
            v2e = [pp.tile([128, R, B, 2, 65], BF16, name=f"v2e_{i}") for i in range(2)]
            v2o = [pp.tile([128, R, B, 2, 65], BF16, name=f"v2o_{i}") for i in range(2)]
            ones_sb = pp.tile([128, 64], BF16)
            # V' pack staging, one per dt, ones columns memset once
            sbv = [pp.tile([128, 2, 4, 65], BF16, name=f"sbv_{d}") for d in range(CT)]
            # fp32 partials of the output projection (pass A: dt 0..5)
            oA = [pp.tile([128, 512], F32, name=f"oA_{t}") for t in range(8)]
            nc.vector.memset(ones_sb[:], 1.0)
            for d in range(CT):
                nc.vector.memset(sbv[d][:, :, :, 64:65], 1.0)

            # priority-ordered input loads, all on one ring so chunk 0's
            # K/V weights get full HBM bandwidth first
            def load_w(h):
                s, e = CH_DT[h]
                nc.sync.dma_start(
                    wkc[h][:], bass.AP(wk.ap().tensor, s * 1024,
                                       [[CT * D, 128], [1, (e - s) * 1024]]))
                nc.sync.dma_start(
                    wvc[h][:], bass.AP(wv.ap().tensor, s * 1024,
                                       [[CT * D, 128], [1, (e - s) * 1024]]))

            nc.sync.dma_start(xT_sb[:], xT.ap())
            load_w(0)
            nc.scalar.dma_start(wq_sb[:], wq.ap())
            nc.scalar.dma_start(wo_sb[:], wo.ap())

            # ---- stage A: K^T and V' projections + pipelined all-gathers ----
            with (
                tc.tile_pool(name="proj_ps", bufs=2, space="PSUM") as proj_ps,
                tc.tile_pool(name="stage", bufs=3) as stage,
            ):
                for ch, (dt_s, dt_e) in enumerate(CH_DT):
                    if ch + 1 < NCH:
                        load_w(ch + 1)
                    ndt = dt_e - dt_s
                    pk_in = cc_in_pack[ch].ap()
                    v_base = ndt * KT_DT
                    # K^T for this chunk's dts
                    for dt in range(dt_s, dt_e):
                        ps = proj_ps.tile([128, ROWS], F32, tag="ps")
                        for ct in range(CT):
                            nc.tensor.matmul(
                                ps[:],
                                wkc[ch][:, dt - dt_s, ct, :],
                                xT_sb[:, ct, :],
                                start=(ct == 0),
                                stop=(ct == CT - 1),
                            )
                        sb = stage.tile([128, ROWS], BF16, tag="kv")
                        nc.scalar.activation(sb[:], ps[:], COPY)
                        nc.sync.dma_start(
                            pack_ap(pk_in, (dt - dt_s) * KT_DT,
                                    [[512, 128], [1, 512]]),
                            sb[:],
                        )
                    # V' for this chunk's dts: per row-block st=(b,j), 2dt of
                    # features at once (N=256), copied into the interleaved
                    # staging tiles feature-slice by feature-slice
                    for st in range(ROWS // 128):
                        b_, j_ = st // 2, st % 2
                        ps = proj_ps.tile([128, ndt * 128], F32, tag="ps")
                        for ct in range(CT):
                            nc.tensor.matmul(
                                ps[:],
                                xT_sb[:, ct, st * 128 : (st + 1) * 128],
                                wvc[ch][:, :, ct, :],
                                start=(ct == 0),
                                stop=(ct == CT - 1),
                            )
                        for dtl in range(ndt):
                            # [128, hp:2, c:64] -> sbv[dt][:, hp, b*2+j, 0:64]
                            nc.vector.tensor_copy(
                                sbv[dt_s + dtl][:, :, b_ * 2 + j_, 0:64],
                                ps[:, dtl * 128 : (dtl + 1) * 128].rearrange(
                                    "p (hp c) -> p hp c", hp=2
                                ),
                            )
                    for dtl in range(ndt):
                        nc.sync.dma_start(
                            pack_ap(pk_in, v_base + dtl * V_DT,
                                    [[520, 128], [1, 520]]),
                            sbv[dt_s + dtl][:],
                        )
                    nc.gpsimd.collective_compute(
                        "AllGather",
                        mybir.AluOpType.bypass,
                        replica_groups=groups,
                        ins=[cc_in_pack[ch].ap().opt()],
                        outs=[cc_out_pack[ch].ap().opt()],
                    )

                # ---- stage B: Q^T projection (overlaps the collectives) ----
                for dt in range(CT):
                    ps = proj_ps.tile([128, ROWS], F32, tag="ps")
                    for ct in range(CT):
                        nc.tensor.matmul(
                            ps[:],
                            wq_sb[:, dt, ct, :],
                            xT_sb[:, ct, :],
                            start=(ct == 0),
                            stop=(ct == CT - 1),
                        )
                    nc.scalar.activation(qt_sb[:, dt, :], ps[:], COPY)

            # gathered pack reads (rank r block at r*PACK_ch)
            def kt_src(ch, ddl):
                ndt = CH_DT[ch][1] - CH_DT[ch][0]
                return bass.AP(
                    cc_out_pack[ch].ap().tensor,
                    ddl * KT_DT,
                    [[512, 128], [ndt * PACK_DT, R], [1, 512]],
                )

            def v_src(ch, ddl, hp):
                ndt = CH_DT[ch][1] - CH_DT[ch][0]
                return bass.AP(
                    cc_out_pack[ch].ap().tensor,
                    ndt * KT_DT + ddl * V_DT + hp * 260,
                    [[520, 128], [ndt * PACK_DT, R], [1, 260]],
                )

            def issue_loads(dt):
                # rank-halved loads: the kt loop consumes keys r-major, so
                # scoring starts as soon as ranks 0-3 land even while the
                # second half is still contending with collective HBM traffic
                ch = CH_OF[dt]
                ddl = dt - CH_DT[ch][0]
                ndt = CH_DT[ch][1] - CH_DT[ch][0]
                ks = kt_src(ch, ddl)
                ve_ap = v2e[dt % 2][:].rearrange("p r b j c -> p r (b j c)")
                vo_ap = v2o[dt % 2][:].rearrange("p r b j c -> p r (b j c)")
                vs0 = v_src(ch, ddl, 0)
                vs1 = v_src(ch, ddl, 1)
                for h in range(2):
                    rs = slice(h * 4, (h + 1) * 4)
                    roff = h * 4 * ndt * PACK_DT
                    nc.gpsimd.dma_start(
                        kt2[dt % 2][:, rs, :],
                        bass.AP(ks.tensor, ks.offset + roff,
                                [ks.ap[0], [ks.ap[1][0], 4], ks.ap[2]]),
                    )
                    nc.gpsimd.dma_start(
                        ve_ap[:, rs, :],
                        bass.AP(vs0.tensor, vs0.offset + roff,
                                [vs0.ap[0], [vs0.ap[1][0], 4], vs0.ap[2]]),
                    )
                    nc.gpsimd.dma_start(
                        vo_ap[:, rs, :],
                        bass.AP(vs1.tensor, vs1.offset + roff,
                                [vs1.ap[0], [vs1.ap[1][0], 4], vs1.ap[2]]),
                    )

            # ---- attention: 8 groups of (2 heads x 2 batches) ----
            issue_loads(0)
            issue_loads(1)
            with (
                tc.tile_pool(name="att_ps", bufs=2, space="PSUM") as att_psp,
                tc.tile_pool(name="pt", bufs=8) as ptp,
                tc.tile_pool(name="rec", bufs=4) as recp,
            ):
                for dt in range(CT):
                    k2 = kt2[dt % 2]
                    ve = v2e[dt % 2]
                    vo = v2o[dt % 2]
                    # one accumulator tile per (b, hp): separate tiles so each
                    # accumulation chain owns its PSUM bank (start=True clears
                    # has_written at bank granularity — chains must not share)
                    at_ps = [
                        [att_psp.tile([128, SQ], F32, tag="at", bufs=4,
                                      name=f"at_{dt}_{b}_{hp}")
                         for hp in range(2)]
                        for b in range(B)
                    ]
                    # software-pipelined kt loop: the PE queue is in-order, so
                    # PV(kt) issued right after scores(kt) head-blocks the
                    # queue on exp(kt). Delay each kt's PV matmuls until after
                    # the NEXT kt's scores have been issued — the PE streams
                    # scores(kt+1) while the exps of kt run.
                    def issue_pv(kt, pts):
                        rr, jh = kt // 2, kt % 2
                        for hp in range(2):
                            vt = ve if hp == 0 else vo
                            for b in range(B):
                                nc.tensor.matmul(
                                    at_ps[b][hp][0:65, :],
                                    vt[:, rr, b, jh, 0:65],
                                    pts[hp][:, b * SQ : (b + 1) * SQ],
                                    start=(kt == 0),
                                    stop=(kt == NKT - 1),
                                )

                    pipe = []
                    for kt in range(NKT):
                        rr, jh = kt // 2, kt % 2
                        pts = []
                        for hp in range(2):
                            hs = slice(hp * 64, (hp + 1) * 64)
                            st2 = att_psp.tile([128, 2 * SQ], F32, tag="st", bufs=4)
                            for b in range(B):
                                nc.tensor.matmul(
                                    st2[:, b * SQ : (b + 1) * SQ],
                                    k2[hs, rr, b * SQ + jh * 128 : b * SQ + jh * 128 + 128],
                                    qt_sb[hs, dt, b * SQ : (b + 1) * SQ],
                                    start=True,
                                    stop=True,
                                )
                            pt2 = ptp.tile([128, 2 * SQ], BF16, tag="pt")
                            if (kt * 2 + hp) % 32 in DVE_HALF:
                                nc.vector.tensor_scalar(
                                    pt2[:].bitcast(I16),
                                    st2[:],
                                    SCH_A,
                                    SCH_B,
                                    MULT,
                                    ADD,
                                )
                            else:
                                nc.scalar.activation(pt2[:], st2[:], EXP)
                            pts.append(pt2)
                        pipe.append(pts)
                        if kt >= 2:
                            # PV lags TWO kts behind scores: its exp is long
                            # done, so the in-order PE queue never waits
                            issue_pv(kt - 2, pipe[kt - 2])
                    issue_pv(NKT - 2, pipe[NKT - 2])
                    issue_pv(NKT - 1, pipe[NKT - 1])
                    # prefetch dt+2's K^T/V' — issued AFTER this dt's last
                    # reads of the shared (dt%2)-parity buffers so the tile
                    # scheduler sees it as WAR (write waits for our reads),
                    # not RAW; it executes during dt+1's compute
                    if dt + 2 < CT:
                        issue_loads(dt + 2)
                    # normalize by the softmax sums (partition 64 of each
                    # accumulator): cast sums to bf16, broadcast across 64
                    # partitions with a 1-row ones-matmul on PE, reciprocal,
                    # multiply.
                    for b in range(B):
                        bcol = b * SQ
                        for hp in range(2):
                            ps = at_ps[b][hp]
                            sums = recp.tile([128, SQ], BF16, tag="sums")
                            bc_ps = att_psp.tile([64, SQ], F32, tag="st", bufs=4,
                                                 name=f"bc_{dt}_{b}_{hp}")
                            bc_sb = recp.tile([64, SQ], F32, tag="bcs")
                            nc.scalar.activation(sums[64:65, :], ps[64:65, :], COPY)
                            nc.tensor.matmul(
                                bc_ps[:],
                                ones_sb[64:65, :],
                                sums[64:65, :],
                                start=True,
                                stop=True,
                            )
                            nc.vector.reciprocal_approx_fast(bc_sb[:], bc_ps[:])
                            if hp == 0:
                                nc.vector.tensor_mul(
                                    at_sb[0:64, dt, bcol : bcol + SQ],
                                    ps[0:64, :],
                                    bc_sb[:],
                                )
                            else:
                                shift = recp.tile([64, SQ], BF16, tag="shift")
                                nc.vector.tensor_mul(shift[:], ps[0:64, :], bc_sb[:])
                                nc.sync.dma_start(
                                    at_sb[64:128, dt, bcol : bcol + SQ], shift[:]
                                )
                    if dt == 5:
                        # ---- output projection pass A: contract dt 0..5 of
                        # at_sb into fp32 SBUF partials inside the window
                        # where dt6/dt7 usually wait on the final all-gather
                        # chunk; borrows the idle "st" PSUM slots
                        for t in range(8):
                            st_, nh = t // 2, t % 2
                            ps = att_psp.tile([128, 512], F32, tag="st", bufs=4,
                                              name=f"oA_ps_{t}")
                            for d in range(6):
                                nc.tensor.matmul(
                                    ps[:],
                                    at_sb[:, d, st_ * 128 : (st_ + 1) * 128],
                                    wo_sb[:, d, nh * 512 : (nh + 1) * 512],
                                    start=(d == 0),
                                    stop=(d == 5),
                                )
                            nc.scalar.activation(oA[t][:], ps[:], COPY)

            if debug:
                nc.sync.dma_start(dbg["qt"].ap(), qt_sb[:])
                nc.sync.dma_start(dbg["at"].ap(), at_sb[:])
                nc.sync.dma_start(dbg["kt"].ap(), kt2[0][:])
                nc.sync.dma_start(
                    dbg["ve"].ap(), v2e[0][:].rearrange("p r b j c -> p (r b j c)")
                )
                nc.sync.dma_start(
                    dbg["vo"].ap(), v2o[0][:].rearrange("p r b j c -> p (r b j c)")
                )

            # ---- output projection pass B: last two head-pairs + add + store
            with (
                tc.tile_pool(name="oB_ps", bufs=3, space="PSUM") as obp,
                tc.tile_pool(name="oB_sb", bufs=3) as obs,
            ):
                for t in range(8):
                    st_, nh = t // 2, t % 2
                    ps = obp.tile([128, 512], F32, tag="oB")
                    for d in (6, 7):
                        nc.tensor.matmul(
                            ps[:],
                            at_sb[:, d, st_ * 128 : (st_ + 1) * 128],
                            wo_sb[:, d, nh * 512 : (nh + 1) * 512],
                            start=(d == 6),
                            stop=(d == 7),
                        )
                    osb = obs.tile([128, 512], F32, tag="os")
                    nc.vector.tensor_add(osb[:], ps[:], oA[t][:])
                    nc.sync.dma_start(
                        out[st_ * 128 : (st_ + 1) * 128, nh * 512 : (nh + 1) * 512],
                        osb[:],
                    )

    nc.compile()
    return nc


_NC = None


def _get_nc():
    global _NC
    if _NC is None:
        _NC = build_graph()
    return _NC


def _warr(w):
    # [d_in, d_out] -> [p, ct, d_out] flattened to [128, CT*D] (contiguous load)
    return np.ascontiguousarray(
        np.asarray(w, np.float32).reshape(CT, 128, D).transpose(1, 0, 2)
    ).astype(NP_BF16).reshape(128, CT * D)


def _warr_dt(w):
    # [d_in, d_out] -> [p, dt, ct, c] flattened (dt-major: chunk 0 loads first)
    return np.ascontiguousarray(
        np.asarray(w, np.float32).reshape(CT, 128, CT, 128).transpose(1, 2, 0, 3)
    ).astype(NP_BF16).reshape(128, CT * D)


def make_in_maps(x, W_q, W_k, W_v, W_o):
    wq = _warr_dt(np.asarray(W_q, np.float32) * 0.125)
    wk = _warr_dt(W_k)
    wv = _warr_dt(W_v)
    wo = _warr(W_o)
    x = np.asarray(x, np.float32)
    in_maps = []
    for r in range(R):
        shard = x[:, r * SQ : (r + 1) * SQ, :].reshape(ROWS, D)  # batch-major rows
        xT_r = np.ascontiguousarray(
            shard.T.reshape(CT, 128, ROWS).transpose(1, 0, 2)
        ).astype(NP_BF16).reshape(128, CT * ROWS)
        in_maps.append({"xT": xT_r, "wq": wq, "wk": wk, "wv": wv, "wo": wo})
    return in_maps


def assemble_out(results):
    full = np.zeros((B, S, D), np.float32)
    for r in range(R):
        o = np.asarray(results[r]["out"], np.float32)
        for b in range(B):
            full[b, r * SQ : (r + 1) * SQ, :] = o[b * SQ : (b + 1) * SQ, :]
    return full


def run(x, W_q, W_k, W_v, W_o, trace=False):
    nc = _get_nc()
    in_maps = make_in_maps(x, W_q, W_k, W_v, W_o)
    res = run_bass_kernel_spmd(nc, in_maps, core_ids=list(range(R)), trace=trace)
    return assemble_out(res.results), res


def kernel(x, W_q, W_k, W_v, W_o):
    out, _ = run(x, W_q, W_k, W_v, W_o)
    return out


# revision 39
# speedup vs baseline: 1.3520x; 1.0085x over previous
"""Multi-head attention forward, distributed over 8 TRN2 NeuronCores.

Sharding: sequence-parallel. Each core owns S/8 = 256 query rows per batch
(512 rows total, batch-major). It computes K^T and V' projections for its own
row shard, all-gathers K^T and V' across the 8 cores in four pipelined
2-head-pair chunks (each triggered as soon as its projections land), then
computes all 16 heads of attention for its query rows plus the output
projection — output rows are disjoint across cores, so there is no reduce at
the end.

Everything on-device stays in the "transposed" layout (feature dim on
partitions) so no transposes are ever needed:
  QT/KT: [d, s]  (d on partitions)    scores^T: [keys, queries]
  V':    [s, d]  (keys on partitions) attn_out^T: [d, queries]

V' ships through the all-gather already in its PV-ready interleaved layout
[key, (b, j, feat0..63, ones)] — the softmax-denominator ones column is baked
into the pack on the producer side, so the consumer needs exactly one
contiguous DMA per (dt, head-half) and the PV matmul (M=65) emits the
denominator on partition 64 for free.

Softmax: scores are bounded (|s| < 9 measured), so exp() without
max-subtraction is safe. exp is split across two engines per key-tile:
ACT runs the LUT exp; DVE runs a Schraudolph bit-trick exp directly in bf16
(one scalar_tensor_tensor emitting int16 bits: y = floor(x*128*log2e + B),
reinterpreted as bf16; rms rel err ~1.8% on the DVE-assigned tiles).

Output projection is split: head-pairs 0..6 are contracted into fp32 SBUF
partials while the tail of attention still runs; only head-pair 7's matmul,
one add, and the store remain after attention.

Compute dtype bf16 (fp32 PSUM accumulation).
"""

import sys

sys.path.insert(0, "/opt/trn_rl_repo")

import numpy as np
import ml_dtypes

import concourse.bass as bass
import concourse.mybir as mybir
import concourse.tile as tile
from concourse import bacc
from concourse.bass_utils import run_bass_kernel_spmd

R = 8          # cores
B = 2
S = 2048
D = 1024
H = 16
DK = 64
SQ = S // R    # 256 queries per batch per core
ROWS = B * SQ  # 512 rows per core, batch-major
CT = D // 128  # 8 contraction tiles
NKT = S // 128  # 16 key tiles per batch

BF16 = mybir.dt.bfloat16
F32 = mybir.dt.float32
I16 = mybir.dt.int16
EXP = mybir.ActivationFunctionType.Exp
COPY = mybir.ActivationFunctionType.Copy
MULT = mybir.AluOpType.mult
ADD = mybir.AluOpType.add
NP_BF16 = ml_dtypes.bfloat16

# Schraudolph bf16 exp: bits = floor(x * 128*log2e + SCH_B), viewed as bf16.
# DVE f32->int16 conversion truncates (measured), so SCH_B is calibrated for
# floor semantics (c = 6.5).
SCH_A = 128.0 * 1.4426950408889634
SCH_B = 127.0 * 128.0 - 6.5
# half-tiles (kt*2+hp) handled by the DVE exp (rest go to ACT); 16 of 32 —
# all odd halves, so every kt runs one ACT exp and one DVE exp concurrently
DVE_HALF = frozenset(range(1, 32, 2))

# all-gather chunking: equal chunks so the tail head-pairs' data arrives
# before the attention wavefront reaches them
CH_DT = [(0, 2), (2, 4), (4, 6), (6, 8)]
CH_OF = {dt: ch for ch, (s, e) in enumerate(CH_DT) for dt in range(s, e)}
NCH = len(CH_DT)
KT_DT = 128 * 512            # KT pack elements per dt
V_DT = 128 * 520             # V' pack elements per dt (2 hp * 4 slots * 65)
PACK_DT = KT_DT + V_DT


def build_graph(debug=False):
    nc = bacc.Bacc(None, target_bir_lowering=False, num_devices=R)

    # inputs arrive pre-arranged on the host to the exact SBUF layouts
    # ([p, ct, ...] with p the partition), so every load is contiguous
    xT = nc.declare_dram_parameter("xT", [128, CT * ROWS], BF16, isOutput=False)
    # wq/wk/wv are dt-major ([p, dt, ct, 128]) so chunk 0's slices load first
    wq = nc.declare_dram_parameter("wq", [128, CT * D], BF16, isOutput=False)
    wk = nc.declare_dram_parameter("wk", [128, CT * D], BF16, isOutput=False)
    wv = nc.declare_dram_parameter("wv", [128, CT * D], BF16, isOutput=False)
    wo = nc.declare_dram_parameter("wo", [128, CT * D], BF16, isOutput=False)
    out = nc.declare_dram_parameter("out", [ROWS, D], F32, isOutput=True)

    # Per-chunk packed bounce buffers.
    # KT region (per dt): flat d_local*512 + s with d_local = p.
    # V' region (per dt): flat p*520 + hp*260 + b*130 + j*65 + c, where the
    # batch-b key is k = r*256 + j*128 + p, feature d = dt*128 + hp*64 + c for
    # c in [0,64), and c = 64 is the constant-ones softmax column.
    cc_in_pack = [
        nc.dram_tensor(f"cc_in_pack{h}", [(e - s) * PACK_DT // 256, 256], BF16)
        for h, (s, e) in enumerate(CH_DT)
    ]
    cc_out_pack = [
        nc.dram_tensor(
            f"cc_out_pack{h}", [R * (e - s) * PACK_DT // 256, 256], BF16,
            addr_space="Shared",
        )
        for h, (s, e) in enumerate(CH_DT)
    ]
    groups = [list(range(R))]

    dbg = {}
    if debug:
        dbg["qt"] = nc.declare_dram_parameter("dbg_qt", [128, CT * ROWS], BF16, isOutput=True)
        dbg["at"] = nc.declare_dram_parameter("dbg_at", [128, CT * ROWS], BF16, isOutput=True)
        dbg["kt"] = nc.declare_dram_parameter("dbg_kt", [128, R * ROWS], BF16, isOutput=True)
        dbg["ve"] = nc.declare_dram_parameter("dbg_ve", [128, R * B * 2 * 65], BF16, isOutput=True)
        dbg["vo"] = nc.declare_dram_parameter("dbg_vo", [128, R * B * 2 * 65], BF16, isOutput=True)

    def pack_ap(tensor_ap, offset, dims):
        return bass.AP(tensor_ap.tensor, offset, dims)

    with tile.TileContext(nc) as tc:
        with tc.tile_pool(name="persist", bufs=1) as pp:
            xT_sb = pp.tile([128, CT, ROWS], BF16)
            wq_sb = pp.tile([128, CT, CT, 128], BF16)
            wkc = [pp.tile([128, e - s, CT, 128], BF16, name=f"wkc{h}")
                   for h, (s, e) in enumerate(CH_DT)]
            wvc = [pp.tile([128, e - s, CT, 128], BF16, name=f"wvc{h}")
                   for h, (s, e) in enumerate(CH_DT)]
            wo_sb = pp.tile([128, CT, D], BF16)
            qt_sb = pp.tile([128, CT, ROWS], BF16)
            at_sb = pp.tile([128, CT, ROWS], BF16)
            # double-buffered attention inputs, one buffer pair per dt parity
            kt2 = [pp.tile([128, R, ROWS], BF16, name=f"kt2_{i}") for i in range(2)]
            # V' per (r, b, j) slot: [data(64) | ones(1)]; ones arrive via AG
            v2e = [pp.tile([128, R, B, 2, 65], BF16, name=f"v2e_{i}") for i in range(2)]
            v2o = [pp.tile([128, R, B, 2, 65], BF16, name=f"v2o_{i}") for i in range(2)]
            ones_sb = pp.tile([128, 64], BF16)
            # V' pack staging, one per dt, ones columns memset once
            sbv = [pp.tile([128, 2, 4, 65], BF16, name=f"sbv_{d}") for d in range(CT)]
            # fp32 partials of the output projection (pass A: dt 0..5)
            oA = [pp.tile([128, 512], F32, name=f"oA_{t}") for t in range(8)]
            nc.vector.memset(ones_sb[:], 1.0)
            for d in range(CT):
                nc.vector.memset(sbv[d][:, :, :, 64:65], 1.0)

            # priority-ordered input loads, all on one ring so chunk 0's
            # K/V weights get full HBM bandwidth first
            def load_w(h):
                s, e = CH_DT[h]
                nc.sync.dma_start(
                    wkc[h][:], bass.AP(wk.ap().tensor, s * 1024,
                                       [[CT * D, 128], [1, (e - s) * 1024]]))
                nc.sync.dma_start(
                    wvc[h][:], bass.AP(wv.ap().tensor, s * 1024,
                                       [[CT * D, 128], [1, (e - s) * 1024]]))

            nc.sync.dma_start(xT_sb[:], xT.ap())
            load_w(0)
            nc.scalar.dma_start(wq_sb[:], wq.ap())
            nc.scalar.dma_start(wo_sb[:], wo.ap())

            # ---- stage A: K^T and V' projections + pipelined all-gathers ----
            with (
                tc.tile_pool(name="proj_ps", bufs=2, space="PSUM") as proj_ps,
                tc.tile_pool(name="stage", bufs=3) as stage,
            ):
                for ch, (dt_s, dt_e) in enumerate(CH_DT):
                    if ch + 1 < NCH:
                        load_w(ch + 1)
                    ndt = dt_e - dt_s
                    pk_in = cc_in_pack[ch].ap()
                    v_base = ndt * KT_DT
                    # K^T for this chunk's dts
                    for dt in range(dt_s, dt_e):
                        ps = proj_ps.tile([128, ROWS], F32, tag="ps")
                        for ct in range(CT):
                            nc.tensor.matmul(
                                ps[:],
                                wkc[ch][:, dt - dt_s, ct, :],
                                xT_sb[:, ct, :],
                                start=(ct == 0),
                                stop=(ct == CT - 1),
                            )
                        sb = stage.tile([128, ROWS], BF16, tag="kv")
                        nc.scalar.activation(sb[:], ps[:], COPY)
                        nc.sync.dma_start(
                            pack_ap(pk_in, (dt - dt_s) * KT_DT,
                                    [[512, 128], [1, 512]]),
                            sb[:],
                        )
                    # V' for this chunk's dts: per row-block st=(b,j), 2dt of
                    # features at once (N=256), copied into the interleaved
                    # staging tiles feature-slice by feature-slice
                    for st in range(ROWS // 128):
                        b_, j_ = st // 2, st % 2
                        ps = proj_ps.tile([128, ndt * 128], F32, tag="ps")
                        for ct in range(CT):
                            nc.tensor.matmul(
                                ps[:],
                                xT_sb[:, ct, st * 128 : (st + 1) * 128],
                                wvc[ch][:, :, ct, :],
                                start=(ct == 0),
                                stop=(ct == CT - 1),
                            )
                        for dtl in range(ndt):
                            # [128, hp:2, c:64] -> sbv[dt][:, hp, b*2+j, 0:64]
                            nc.vector.tensor_copy(
                                sbv[dt_s + dtl][:, :, b_ * 2 + j_, 0:64],
                                ps[:, dtl * 128 : (dtl + 1) * 128].rearrange(
                                    "p (hp c) -> p hp c", hp=2
                                ),
                            )
                    for dtl in range(ndt):
                        nc.sync.dma_start(
                            pack_ap(pk_in, v_base + dtl * V_DT,
                                    [[520, 128], [1, 520]]),
                            sbv[dt_s + dtl][:],
                        )
                    nc.gpsimd.collective_compute(
                        "AllGather",
                        mybir.AluOpType.bypass,
                        replica_groups=groups,
                        ins=[cc_in_pack[ch].ap().opt()],
                        outs=[cc_out_pack[ch].ap().opt()],
                    )

                # ---- stage B: Q^T projection (overlaps the collectives) ----
                for dt in range(CT):
                    ps = proj_ps.tile([128, ROWS], F32, tag="ps")
                    for ct in range(CT):
                        nc.tensor.matmul(
                            ps[:],
                            wq_sb[:, dt, ct, :],
                            xT_sb[:, ct, :],
                            start=(ct == 0),
                            stop=(ct == CT - 1),
                        )
                    nc.scalar.activation(qt_sb[:, dt, :], ps[:], COPY)

            # gathered pack reads (rank r block at r*PACK_ch)
            def kt_src(ch, ddl):
                ndt = CH_DT[ch][1] - CH_DT[ch][0]
                return bass.AP(
                    cc_out_pack[ch].ap().tensor,
                    ddl * KT_DT,
                    [[512, 128], [ndt * PACK_DT, R], [1, 512]],
                )

            def v_src(ch, ddl, hp):
                ndt = CH_DT[ch][1] - CH_DT[ch][0]
                return bass.AP(
                    cc_out_pack[ch].ap().tensor,
                    ndt * KT_DT + ddl * V_DT + hp * 260,
                    [[520, 128], [ndt * PACK_DT, R], [1, 260]],
                )

            def issue_loads(dt):
                # rank-halved loads: the kt loop consumes keys r-major, so
                # scoring starts as soon as ranks 0-3 land even while the
                # second half is still contending with collective HBM traffic
                ch = CH_OF[dt]
                ddl = dt - CH_DT[ch][0]
                ndt = CH_DT[ch][1] - CH_DT[ch][0]
                ks = kt_src(ch, ddl)
                ve_ap = v2e[dt % 2][:].rearrange("p r b j c -> p r (b j c)")
                vo_ap = v2o[dt % 2][:].rearrange("p r b j c -> p r (b j c)")
                vs0 = v_src(ch, ddl, 0)
                vs1 = v_src(ch, ddl, 1)
                for h in range(2):
                    rs = slice(h * 4, (h + 1) * 4)
                    roff = h * 4 * ndt * PACK_DT
                    nc.gpsimd.dma_start(
                        kt2[dt % 2][:, rs, :],
                        bass.AP(ks.tensor, ks.offset + roff,
                                [ks.ap[0], [ks.ap[1][0], 4], ks.ap[2]]),
                    )
                    nc.gpsimd.dma_start(
                        ve_ap[:, rs, :],
                        bass.AP(vs0.tensor, vs0.offset + roff,
                                [vs0.ap[0], [vs0.ap[1][0], 4], vs0.ap[2]]),
                    )
                    nc.gpsimd.dma_start(
                        vo_ap[:, rs, :],
                        bass.AP(vs1.tensor, vs1.offset + roff,
                                [vs1.ap[0], [vs1.ap[1][0], 4], vs1.ap[2]]),
                    )

            # ---- attention: 8 groups of (2 heads x 2 batches) ----
            issue_loads(0)
            issue_loads(1)
            with (
                tc.tile_pool(name="att_ps", bufs=2, space="PSUM") as att_psp,
                tc.tile_pool(name="pt", bufs=8) as ptp,
                tc.tile_pool(name="rec", bufs=4) as recp,
            ):
                for dt in range(CT):
                    k2 = kt2[dt % 2]
                    ve = v2e[dt % 2]
                    vo = v2o[dt % 2]
                    # one accumulator tile per (b, hp): separate tiles so each
                    # accumulation chain owns its PSUM bank (start=True clears
                    # has_written at bank granularity — chains must not share)
                    at_ps = [
                        [att_psp.tile([128, SQ], F32, tag="at", bufs=4,
                                      name=f"at_{dt}_{b}_{hp}")
                         for hp in range(2)]
                        for b in range(B)
                    ]
                    # software-pipelined kt loop: the PE queue is in-order, so
                    # PV(kt) issued right after scores(kt) head-blocks the
                    # queue on exp(kt). Delay each kt's PV matmuls until after
                    # the NEXT kt's scores have been issued — the PE streams
                    # scores(kt+1) while the exps of kt run.
                    def issue_pv(kt, pts):
                        rr, jh = kt // 2, kt % 2
                        for hp in range(2):
                            vt = ve if hp == 0 else vo
                            for b in range(B):
                                nc.tensor.matmul(
                                    at_ps[b][hp][0:65, :],
                                    vt[:, rr, b, jh, 0:65],
                                    pts[hp][:, b * SQ : (b + 1) * SQ],
                                    start=(kt == 0),
                                    stop=(kt == NKT - 1),
                                )

                    pipe = []
                    for kt in range(NKT):
                        rr, jh = kt // 2, kt % 2
                        pts = []
                        for hp in range(2):
                            hs = slice(hp * 64, (hp + 1) * 64)
                            st2 = att_psp.tile([128, 2 * SQ], F32, tag="st", bufs=4)
                            for b in range(B):
                                nc.tensor.matmul(
                                    st2[:, b * SQ : (b + 1) * SQ],
                                    k2[hs, rr, b * SQ + jh * 128 : b * SQ + jh * 128 + 128],
                                    qt_sb[hs, dt, b * SQ : (b + 1) * SQ],
                                    start=True,
                                    stop=True,
                                )
                            pt2 = ptp.tile([128, 2 * SQ], BF16, tag="pt")
                            if (kt * 2 + hp) % 32 in DVE_HALF:
                                nc.vector.tensor_scalar(
                                    pt2[:].bitcast(I16),
                                    st2[:],
                                    SCH_A,
                                    SCH_B,
                                    MULT,
                                    ADD,
                                )
                            else:
                                nc.scalar.activation(pt2[:], st2[:], EXP)
                            pts.append(pt2)
                        pipe.append(pts)
                        if kt >= 2:
                            # PV lags TWO kts behind scores: its exp is long
                            # done, so the in-order PE queue never waits
                            issue_pv(kt - 2, pipe[kt - 2])
                    issue_pv(NKT - 2, pipe[NKT - 2])
                    issue_pv(NKT - 1, pipe[NKT - 1])
                    # prefetch dt+2's K^T/V' — issued AFTER this dt's last
                    # reads of the shared (dt%2)-parity buffers so the tile
                    # scheduler sees it as WAR (write waits for our reads),
                    # not RAW; it executes during dt+1's compute
                    if dt + 2 < CT:
                        issue_loads(dt + 2)
                    # normalize by the softmax sums (partition 64 of each
                    # accumulator): cast sums to bf16, broadcast across 64
                    # partitions with a 1-row ones-matmul on PE, reciprocal,
                    # multiply.
                    for b in range(B):
                        bcol = b * SQ
                        for hp in range(2):
                            ps = at_ps[b][hp]
                            sums = recp.tile([128, SQ], BF16, tag="sums")
                            bc_ps = att_psp.tile([64, SQ], F32, tag="st", bufs=4,
                                                 name=f"bc_{dt}_{b}_{hp}")
                            bc_sb = recp.tile([64, SQ], F32, tag="bcs")
                            nc.scalar.activation(sums[64:65, :], ps[64:65, :], COPY)
                            nc.tensor.matmul(
                                bc_ps[:],
                                ones_sb[64:65, :],
                                sums[64:65, :],
                                start=True,
                                stop=True,
                            )
                            nc.vector.reciprocal_approx_fast(bc_sb[:], bc_ps[:])
                            if hp == 0:
                                nc.vector.tensor_mul(
                                    at_sb[0:64, dt, bcol : bcol + SQ],
                                    ps[0:64, :],
                                    bc_sb[:],
                                )
                            else:
                                shift = recp.tile([64, SQ], BF16, tag="shift")
                                nc.vector.tensor_mul(shift[:], ps[0:64, :], bc_sb[:])
                                nc.sync.dma_start(
                                    at_sb[64:128, dt, bcol : bcol + SQ], shift[:]
                                )
                    if dt == 5:
                        # ---- output projection pass A: contract dt 0..5 of
                        # at_sb into fp32 SBUF partials inside the window
                        # where dt6/dt7 usually wait on the final all-gather
                        # chunk; borrows the idle "st" PSUM slots
                        for t in range(8):
                            st_, nh = t // 2, t % 2
                            ps = att_psp.tile([128, 512], F32, tag="st", bufs=4,
                                              name=f"oA_ps_{t}")
                            for d in range(6):
                                nc.tensor.matmul(
                                    ps[:],
                                    at_sb[:, d, st_ * 128 : (st_ + 1) * 128],
                                    wo_sb[:, d, nh * 512 : (nh + 1) * 512],
                                    start=(d == 0),
                                    stop=(d == 5),
                                )
                            nc.scalar.activation(oA[t][:], ps[:], COPY)

            if debug:
                nc.sync.dma_start(dbg["qt"].ap(), qt_sb[:])
                nc.sync.dma_start(dbg["at"].ap(), at_sb[:])
                nc.sync.dma_start(dbg["kt"].ap(), kt2[0][:])
                nc.sync.dma_start(
                    dbg["ve"].ap(), v2e[0][:].rearrange("p r b j c -> p (r b j c)")
                )
                nc.sync.dma_start(
                    dbg["vo"].ap(), v2o[0][:].rearrange("p r b j c -> p (r b j c)")
                )

            # ---- output projection pass B: last two head-pairs + add + store
            with (
                tc.tile_pool(name="oB_ps", bufs=3, space="PSUM") as obp,
                tc.tile_pool(name="oB_sb", bufs=3) as obs,
            ):
                for t in range(8):
                    st_, nh = t // 2, t % 2
                    ps = obp.tile([128, 512], F32, tag="oB")
                    for d in (6, 7):
                        nc.tensor.matmul(
                            ps[:],
                            at_sb[:, d, st_ * 128 : (st_ + 1) * 128],
                            wo_sb[:, d, nh * 512 : (nh + 1) * 512],
                            start=(d == 6),
                            stop=(d == 7),
                        )
                    osb = obs.tile([128, 512], F32, tag="os")
                    nc.vector.tensor_add(osb[:], ps[:], oA[t][:])
                    nc.sync.dma_start(
                        out[st_ * 128 : (st_ + 1) * 128, nh * 512 : (nh + 1) * 512],
                        osb[:],
                    )

    nc.compile()
    return nc


_NC = None


def _get_nc():
    global _NC
    if _NC is None:
        _NC = build_graph()
    return _NC


def _warr(w):
    # [d_in, d_out] -> [p, ct, d_out] flattened to [128, CT*D] (contiguous load)
    return np.ascontiguousarray(
        np.asarray(w, np.float32).reshape(CT, 128, D).transpose(1, 0, 2)
    ).astype(NP_BF16).reshape(128, CT * D)


def _warr_dt(w):
    # [d_in, d_out] -> [p, dt, ct, c] flattened (dt-major: chunk 0 loads first)
    return np.ascontiguousarray(
        np.asarray(w, np.float32).reshape(CT, 128, CT, 128).transpose(1, 2, 0, 3)
    ).astype(NP_BF16).reshape(128, CT * D)


def make_in_maps(x, W_q, W_k, W_v, W_o):
    wq = _warr_dt(np.asarray(W_q, np.float32) * 0.125)
    wk = _warr_dt(W_k)
    wv = _warr_dt(W_v)
    wo = _warr(W_o)
    x = np.asarray(x, np.float32)
    in_maps = []
    for r in range(R):
        shard = x[:, r * SQ : (r + 1) * SQ, :].reshape(ROWS, D)  # batch-major rows
        xT_r = np.ascontiguousarray(
            shard.T.reshape(CT, 128, ROWS).transpose(1, 0, 2)
        ).astype(NP_BF16).reshape(128, CT * ROWS)
        in_maps.append({"xT": xT_r, "wq": wq, "wk": wk, "wv": wv, "wo": wo})
    return in_maps


def assemble_out(results):
    full = np.zeros((B, S, D), np.float32)
    for r in range(R):
        o = np.asarray(results[r]["out"], np.float32)
        for b in range(B):
            full[b, r * SQ : (r + 1) * SQ, :] = o[b * SQ : (b + 1) * SQ, :]
    return full


def run(x, W_q, W_k, W_v, W_o, trace=False):
    nc = _get_nc()
    in_maps = make_in_maps(x, W_q, W_k, W_v, W_o)
    res = run_bass_kernel_spmd(nc, in_maps, core_ids=list(range(R)), trace=trace)
    return assemble_out(res.results), res


def kernel(x, W_q, W_k, W_v, W_o):
    out, _ = run(x, W_q, W_k, W_v, W_o)
    return out


# revision 40
# speedup vs baseline: 1.3635x; 1.0086x over previous
"""Multi-head attention forward, distributed over 8 TRN2 NeuronCores.

Sharding: sequence-parallel. Each core owns S/8 = 256 query rows per batch
(512 rows total, batch-major). It computes K^T and V' projections for its own
row shard, all-gathers K^T and V' across the 8 cores in four pipelined
2-head-pair chunks (each triggered as soon as its projections land), then
computes all 16 heads of attention for its query rows plus the output
projection — output rows are disjoint across cores, so there is no reduce at
the end.

Everything on-device stays in the "transposed" layout (feature dim on
partitions) so no transposes are ever needed:
  QT/KT: [d, s]  (d on partitions)    scores^T: [keys, queries]
  V':    [s, d]  (keys on partitions) attn_out^T: [d, queries]

V' ships through the all-gather already in its PV-ready interleaved layout
[key, (b, j, feat0..63, ones)] — the softmax-denominator ones column is baked
into the pack on the producer side, so the consumer needs exactly one
contiguous DMA per (dt, head-half) and the PV matmul (M=65) emits the
denominator on partition 64 for free.

Softmax: scores are bounded (|s| < 9 measured), so exp() without
max-subtraction is safe. exp is split across two engines per key-tile:
ACT runs the LUT exp; DVE runs a Schraudolph bit-trick exp directly in bf16
(one scalar_tensor_tensor emitting int16 bits: y = floor(x*128*log2e + B),
reinterpreted as bf16; rms rel err ~1.8% on the DVE-assigned tiles).

Output projection is split: head-pairs 0..6 are contracted into fp32 SBUF
partials while the tail of attention still runs; only head-pair 7's matmul,
one add, and the store remain after attention.

Compute dtype bf16 (fp32 PSUM accumulation).
"""

import sys

sys.path.insert(0, "/opt/trn_rl_repo")

import numpy as np
import ml_dtypes

import concourse.bass as bass
import concourse.mybir as mybir
import concourse.tile as tile
from concourse import bacc
from concourse.bass_utils import run_bass_kernel_spmd

R = 8          # cores
B = 2
S = 2048
D = 1024
H = 16
DK = 64
SQ = S // R    # 256 queries per batch per core
ROWS = B * SQ  # 512 rows per core, batch-major
CT = D // 128  # 8 contraction tiles
NKT = S // 128  # 16 key tiles per batch

BF16 = mybir.dt.bfloat16
F32 = mybir.dt.float32
I16 = mybir.dt.int16
EXP = mybir.ActivationFunctionType.Exp
COPY = mybir.ActivationFunctionType.Copy
MULT = mybir.AluOpType.mult
ADD = mybir.AluOpType.add
NP_BF16 = ml_dtypes.bfloat16

# Schraudolph bf16 exp: bits = floor(x * 128*log2e + SCH_B), viewed as bf16.
# DVE f32->int16 conversion truncates (measured), so SCH_B is calibrated for
# floor semantics (c = 6.5).
SCH_A = 128.0 * 1.4426950408889634
SCH_B = 127.0 * 128.0 - 6.5
# half-tiles (kt*2+hp) handled by the DVE exp (rest go to ACT); 16 of 32 —
# all odd halves, so every kt runs one ACT exp and one DVE exp concurrently
DVE_HALF = frozenset(range(1, 32, 2))

# all-gather chunking: equal chunks so the tail head-pairs' data arrives
# before the attention wavefront reaches them
CH_DT = [(0, 2), (2, 4), (4, 6), (6, 8)]
CH_OF = {dt: ch for ch, (s, e) in enumerate(CH_DT) for dt in range(s, e)}
NCH = len(CH_DT)
KT_DT = 128 * 512            # KT pack elements per dt
V_DT = 128 * 520             # V' pack elements per dt (2 hp * 4 slots * 65)
PACK_DT = KT_DT + V_DT


def build_graph(debug=False):
    nc = bacc.Bacc(None, target_bir_lowering=False, num_devices=R)

    # inputs arrive pre-arranged on the host to the exact SBUF layouts
    # ([p, ct, ...] with p the partition), so every load is contiguous
    xT = nc.declare_dram_parameter("xT", [128, CT * ROWS], BF16, isOutput=False)
    # wq/wk/wv are dt-major ([p, dt, ct, 128]) so chunk 0's slices load first
    wq = nc.declare_dram_parameter("wq", [128, CT * D], BF16, isOutput=False)
    wk = nc.declare_dram_parameter("wk", [128, CT * D], BF16, isOutput=False)
    wv = nc.declare_dram_parameter("wv", [128, CT * D], BF16, isOutput=False)
    wo = nc.declare_dram_parameter("wo", [128, CT * D], BF16, isOutput=False)
    out = nc.declare_dram_parameter("out", [ROWS, D], F32, isOutput=True)

    # Per-chunk packed bounce buffers.
    # KT region (per dt): flat d_local*512 + s with d_local = p.
    # V' region (per dt): flat p*520 + hp*260 + b*130 + j*65 + c, where the
    # batch-b key is k = r*256 + j*128 + p, feature d = dt*128 + hp*64 + c for
    # c in [0,64), and c = 64 is the constant-ones softmax column.
    cc_in_pack = [
        nc.dram_tensor(f"cc_in_pack{h}", [(e - s) * PACK_DT // 256, 256], BF16)
        for h, (s, e) in enumerate(CH_DT)
    ]
    cc_out_pack = [
        nc.dram_tensor(
            f"cc_out_pack{h}", [R * (e - s) * PACK_DT // 256, 256], BF16,
            addr_space="Shared",
        )
        for h, (s, e) in enumerate(CH_DT)
    ]
    groups = [list(range(R))]

    dbg = {}
    if debug:
        dbg["qt"] = nc.declare_dram_parameter("dbg_qt", [128, CT * ROWS], BF16, isOutput=True)
        dbg["at"] = nc.declare_dram_parameter("dbg_at", [128, CT * ROWS], BF16, isOutput=True)
        dbg["kt"] = nc.declare_dram_parameter("dbg_kt", [128, R * ROWS], BF16, isOutput=True)
        dbg["ve"] = nc.declare_dram_parameter("dbg_ve", [128, R * B * 2 * 65], BF16, isOutput=True)
        dbg["vo"] = nc.declare_dram_parameter("dbg_vo", [128, R * B * 2 * 65], BF16, isOutput=True)

    def pack_ap(tensor_ap, offset, dims):
        return bass.AP(tensor_ap.tensor, offset, dims)

    with tile.TileContext(nc) as tc:
        with tc.tile_pool(name="persist", bufs=1) as pp:
            xT_sb = pp.tile([128, CT, ROWS], BF16)
            wq_sb = pp.tile([128, CT, CT, 128], BF16)
            wkc = [pp.tile([128, e - s, CT, 128], BF16, name=f"wkc{h}")
                   for h, (s, e) in enumerate(CH_DT)]
            wvc = [pp.tile([128, e - s, CT, 128], BF16, name=f"wvc{h}")
                   for h, (s, e) in enumerate(CH_DT)]
            wo_sb = pp.tile([128, CT, D], BF16)
            qt_sb = pp.tile([128, CT, ROWS], BF16)
            at_sb = pp.tile([128, CT, ROWS], BF16)
            # double-buffered attention inputs, one buffer pair per dt parity
            kt2 = [pp.tile([128, R, ROWS], BF16, name=f"kt2_{i}") for i in range(2)]
            # V' per (r, b, j) slot: [data(64) | ones(1)]; ones arrive via AG
            v2e = [pp.tile([128, R, B, 2, 65], BF16, name=f"v2e_{i}") for i in range(2)]
            v2o = [pp.tile([128, R, B, 2, 65], BF16, name=f"v2o_{i}") for i in range(2)]
            ones_sb = pp.tile([128, 64], BF16)
            # V' pack staging, one per dt, ones columns memset once
            sbv = [pp.tile([128, 2, 4, 65], BF16, name=f"sbv_{d}") for d in range(CT)]
            # fp32 partials of the output projection (pass A: dt 0..5)
            oA = [pp.tile([128, 512], F32, name=f"oA_{t}") for t in range(8)]
            nc.vector.memset(ones_sb[:], 1.0)
            for d in range(CT):
                nc.vector.memset(sbv[d][:, :, :, 64:65], 1.0)

            # priority-ordered input loads, all on one ring so chunk 0's
            # K/V weights get full HBM bandwidth first
            def load_w(h):
                s, e = CH_DT[h]
                nc.sync.dma_start(
                    wkc[h][:], bass.AP(wk.ap().tensor, s * 1024,
                                       [[CT * D, 128], [1, (e - s) * 1024]]))
                nc.sync.dma_start(
                    wvc[h][:], bass.AP(wv.ap().tensor, s * 1024,
                                       [[CT * D, 128], [1, (e - s) * 1024]]))

            nc.sync.dma_start(xT_sb[:], xT.ap())
            load_w(0)
            nc.scalar.dma_start(wq_sb[:], wq.ap())
            nc.scalar.dma_start(wo_sb[:], wo.ap())

            # ---- stage A: K^T and V' projections + pipelined all-gathers ----
            with (
                tc.tile_pool(name="proj_ps", bufs=2, space="PSUM") as proj_ps,
                tc.tile_pool(name="stage", bufs=3) as stage,
            ):
                for ch, (dt_s, dt_e) in enumerate(CH_DT):
                    if ch + 1 < NCH:
                        load_w(ch + 1)
                    ndt = dt_e - dt_s
                    pk_in = cc_in_pack[ch].ap()
                    v_base = ndt * KT_DT
                    # K^T for this chunk's dts
                    for dt in range(dt_s, dt_e):
                        ps = proj_ps.tile([128, ROWS], F32, tag="ps")
                        for ct in range(CT):
                            nc.tensor.matmul(
                                ps[:],
                                wkc[ch][:, dt - dt_s, ct, :],
                                xT_sb[:, ct, :],
                                start=(ct == 0),
                                stop=(ct == CT - 1),
                            )
                        sb = stage.tile([128, ROWS], BF16, tag="kv")
                        nc.scalar.activation(sb[:], ps[:], COPY)
                        nc.sync.dma_start(
                            pack_ap(pk_in, (dt - dt_s) * KT_DT,
                                    [[512, 128], [1, 512]]),
                            sb[:],
                        )
                    # V' for this chunk's dts: per row-block st=(b,j), 2dt of
                    # features at once (N=256), copied into the interleaved
                    # staging tiles feature-slice by feature-slice
                    for st in range(ROWS // 128):
                        b_, j_ = st // 2, st % 2
                        ps = proj_ps.tile([128, ndt * 128], F32, tag="ps")
                        for ct in range(CT):
                            nc.tensor.matmul(
                                ps[:],
                                xT_sb[:, ct, st * 128 : (st + 1) * 128],
                                wvc[ch][:, :, ct, :],
                                start=(ct == 0),
                                stop=(ct == CT - 1),
                            )
                        for dtl in range(ndt):
                            # [128, hp:2, c:64] -> sbv[dt][:, hp, b*2+j, 0:64]
                            nc.vector.tensor_copy(
                                sbv[dt_s + dtl][:, :, b_ * 2 + j_, 0:64],
                                ps[:, dtl * 128 : (dtl + 1) * 128].rearrange(
                                    "p (hp c) -> p hp c", hp=2
                                ),
                            )
                    for dtl in range(ndt):
                        nc.sync.dma_start(
                            pack_ap(pk_in, v_base + dtl * V_DT,
                                    [[520, 128], [1, 520]]),
                            sbv[dt_s + dtl][:],
                        )
                    nc.gpsimd.collective_compute(
                        "AllGather",
                        mybir.AluOpType.bypass,
                        replica_groups=groups,
                        ins=[cc_in_pack[ch].ap().opt()],
                        outs=[cc_out_pack[ch].ap().opt()],
                    )

                # ---- stage B: Q^T projection (overlaps the collectives) ----
                for dt in range(CT):
                    ps = proj_ps.tile([128, ROWS], F32, tag="ps")
                    for ct in range(CT):
                        nc.tensor.matmul(
                            ps[:],
                            wq_sb[:, dt, ct, :],
                            xT_sb[:, ct, :],
                            start=(ct == 0),
                            stop=(ct == CT - 1),
                        )
                    nc.scalar.activation(qt_sb[:, dt, :], ps[:], COPY)

            # gathered pack reads (rank r block at r*PACK_ch)
            def kt_src(ch, ddl):
                ndt = CH_DT[ch][1] - CH_DT[ch][0]
                return bass.AP(
                    cc_out_pack[ch].ap().tensor,
                    ddl * KT_DT,
                    [[512, 128], [ndt * PACK_DT, R], [1, 512]],
                )

            def v_src(ch, ddl, hp):
                ndt = CH_DT[ch][1] - CH_DT[ch][0]
                return bass.AP(
                    cc_out_pack[ch].ap().tensor,
                    ndt * KT_DT + ddl * V_DT + hp * 260,
                    [[520, 128], [ndt * PACK_DT, R], [1, 260]],
                )

            def issue_loads(dt):
                # rank-halved loads: the kt loop consumes keys r-major, so
                # scoring starts as soon as ranks 0-3 land even while the
                # second half is still contending with collective HBM traffic
                ch = CH_OF[dt]
                ddl = dt - CH_DT[ch][0]
                ndt = CH_DT[ch][1] - CH_DT[ch][0]
                ks = kt_src(ch, ddl)
                ve_ap = v2e[dt % 2][:].rearrange("p r b j c -> p r (b j c)")
                vo_ap = v2o[dt % 2][:].rearrange("p r b j c -> p r (b j c)")
                vs0 = v_src(ch, ddl, 0)
                vs1 = v_src(ch, ddl, 1)
                for h in range(2):
                    rs = slice(h * 4, (h + 1) * 4)
                    roff = h * 4 * ndt * PACK_DT
                    nc.gpsimd.dma_start(
                        kt2[dt % 2][:, rs, :],
                        bass.AP(ks.tensor, ks.offset + roff,
                                [ks.ap[0], [ks.ap[1][0], 4], ks.ap[2]]),
                    )
                    nc.gpsimd.dma_start(
                        ve_ap[:, rs, :],
                        bass.AP(vs0.tensor, vs0.offset + roff,
                                [vs0.ap[0], [vs0.ap[1][0], 4], vs0.ap[2]]),
                    )
                    nc.gpsimd.dma_start(
                        vo_ap[:, rs, :],
                        bass.AP(vs1.tensor, vs1.offset + roff,
                                [vs1.ap[0], [vs1.ap[1][0], 4], vs1.ap[2]]),
                    )

            # ---- attention: 8 groups of (2 heads x 2 batches) ----
            issue_loads(0)
            issue_loads(1)
            with (
                tc.tile_pool(name="att_ps", bufs=2, space="PSUM") as att_psp,
                tc.tile_pool(name="pt", bufs=8) as ptp,
                tc.tile_pool(name="rec", bufs=4) as recp,
            ):
                for dt in range(CT):
                    k2 = kt2[dt % 2]
                    ve = v2e[dt % 2]
                    vo = v2o[dt % 2]
                    # one accumulator tile per (b, hp): separate tiles so each
                    # accumulation chain owns its PSUM bank (start=True clears
                    # has_written at bank granularity — chains must not share)
                    at_ps = [
                        [att_psp.tile([128, SQ], F32, tag="at", bufs=4,
                                      name=f"at_{dt}_{b}_{hp}")
                         for hp in range(2)]
                        for b in range(B)
                    ]
                    # software-pipelined kt loop: the PE queue is in-order, so
                    # PV(kt) issued right after scores(kt) head-blocks the
                    # queue on exp(kt). Delay each kt's PV matmuls until after
                    # the NEXT kt's scores have been issued — the PE streams
                    # scores(kt+1) while the exps of kt run.
                    def issue_pv(kt, pts):
                        rr, jh = kt // 2, kt % 2
                        for hp in range(2):
                            vt = ve if hp == 0 else vo
                            for b in range(B):
                                nc.tensor.matmul(
                                    at_ps[b][hp][0:65, :],
                                    vt[:, rr, b, jh, 0:65],
                                    pts[hp][:, b * SQ : (b + 1) * SQ],
                                    start=(kt == 0),
                                    stop=(kt == NKT - 1),
                                )

                    pipe = []
                    for kt in range(NKT):
                        rr, jh = kt // 2, kt % 2
                        pts = []
                        for hp in range(2):
                            hs = slice(hp * 64, (hp + 1) * 64)
                            st2 = att_psp.tile([128, 2 * SQ], F32, tag="st", bufs=4)
                            for b in range(B):
                                nc.tensor.matmul(
                                    st2[:, b * SQ : (b + 1) * SQ],
                                    k2[hs, rr, b * SQ + jh * 128 : b * SQ + jh * 128 + 128],
                                    qt_sb[hs, dt, b * SQ : (b + 1) * SQ],
                                    start=True,
                                    stop=True,
                                )
                            pt2 = ptp.tile([128, 2 * SQ], BF16, tag="pt")
                            if (kt * 2 + hp) % 32 in DVE_HALF:
                                nc.vector.tensor_scalar(
                                    pt2[:].bitcast(I16),
                                    st2[:],
                                    SCH_A,
                                    SCH_B,
                                    MULT,
                                    ADD,
                                )
                            else:
                                nc.scalar.activation(pt2[:], st2[:], EXP)
                            pts.append(pt2)
                        pipe.append(pts)
                        if kt >= 2:
                            # PV lags TWO kts behind scores: its exp is long
                            # done, so the in-order PE queue never waits
                            issue_pv(kt - 2, pipe[kt - 2])
                    issue_pv(NKT - 2, pipe[NKT - 2])
                    issue_pv(NKT - 1, pipe[NKT - 1])
                    # prefetch dt+2's K^T/V' — issued AFTER this dt's last
                    # reads of the shared (dt%2)-parity buffers so the tile
                    # scheduler sees it as WAR (write waits for our reads),
                    # not RAW; it executes during dt+1's compute
                    if dt + 2 < CT:
                        issue_loads(dt + 2)
                    # normalize by the softmax sums (partition 64 of each
                    # accumulator): cast sums to bf16, broadcast across 64
                    # partitions with a 1-row ones-matmul on PE, reciprocal,
                    # multiply.
                    for b in range(B):
                        bcol = b * SQ
                        for hp in range(2):
                            ps = at_ps[b][hp]
                            sums = recp.tile([128, SQ], BF16, tag="sums")
                            bc_ps = att_psp.tile([64, SQ], F32, tag="st", bufs=4,
                                                 name=f"bc_{dt}_{b}_{hp}")
                            bc_sb = recp.tile([64, SQ], F32, tag="bcs")
                            if hp == 0:
                                nc.scalar.activation(sums[64:65, :],
                                                     ps[64:65, :], COPY)
                            else:
                                nc.vector.tensor_copy(sums[64:65, :],
                                                      ps[64:65, :])
                            nc.tensor.matmul(
                                bc_ps[:],
                                ones_sb[64:65, :],
                                sums[64:65, :],
                                start=True,
                                stop=True,
                            )
                            nc.vector.reciprocal_approx_fast(bc_sb[:], bc_ps[:])
                            if hp == 0:
                                nc.vector.tensor_mul(
                                    at_sb[0:64, dt, bcol : bcol + SQ],
                                    ps[0:64, :],
                                    bc_sb[:],
                                )
                            else:
                                shift = recp.tile([64, SQ], BF16, tag="shift")
                                nc.vector.tensor_mul(shift[:], ps[0:64, :], bc_sb[:])
                                nc.sync.dma_start(
                                    at_sb[64:128, dt, bcol : bcol + SQ], shift[:]
                                )
                    if dt == 5:
                        # ---- output projection pass A: contract dt 0..5 of
                        # at_sb into fp32 SBUF partials inside the window
                        # where dt6/dt7 usually wait on the final all-gather
                        # chunk; borrows the idle "st" PSUM slots
                        for t in range(8):
                            st_, nh = t // 2, t % 2
                            ps = att_psp.tile([128, 512], F32, tag="st", bufs=4,
                                              name=f"oA_ps_{t}")
                            for d in range(6):
                                nc.tensor.matmul(
                                    ps[:],
                                    at_sb[:, d, st_ * 128 : (st_ + 1) * 128],
                                    wo_sb[:, d, nh * 512 : (nh + 1) * 512],
                                    start=(d == 0),
                                    stop=(d == 5),
                                )
                            nc.scalar.activation(oA[t][:], ps[:], COPY)

            if debug:
                nc.sync.dma_start(dbg["qt"].ap(), qt_sb[:])
                nc.sync.dma_start(dbg["at"].ap(), at_sb[:])
                nc.sync.dma_start(dbg["kt"].ap(), kt2[0][:])
                nc.sync.dma_start(
                    dbg["ve"].ap(), v2e[0][:].rearrange("p r b j c -> p (r b j c)")
                )
                nc.sync.dma_start(
                    dbg["vo"].ap(), v2o[0][:].rearrange("p r b j c -> p (r b j c)")
                )

            # ---- output projection pass B: last two head-pairs + add + store
            with (
                tc.tile_pool(name="oB_ps", bufs=3, space="PSUM") as obp,
                tc.tile_pool(name="oB_sb", bufs=3) as obs,
            ):
                for t in range(8):
                    st_, nh = t // 2, t % 2
                    ps = obp.tile([128, 512], F32, tag="oB")
                    for d in (6, 7):
                        nc.tensor.matmul(
                            ps[:],
                            at_sb[:, d, st_ * 128 : (st_ + 1) * 128],
                            wo_sb[:, d, nh * 512 : (nh + 1) * 512],
                            start=(d == 6),
                            stop=(d == 7),
                        )
                    osb = obs.tile([128, 512], F32, tag="os")
                    nc.vector.tensor_add(osb[:], ps[:], oA[t][:])
                    nc.sync.dma_start(
                        out[st_ * 128 : (st_ + 1) * 128, nh * 512 : (nh + 1) * 512],
                        osb[:],
                    )

    nc.compile()
    return nc


_NC = None


def _get_nc():
    global _NC
    if _NC is None:
        _NC = build_graph()
    return _NC


def _warr(w):
    # [d_in, d_out] -> [p, ct, d_out] flattened to [128, CT*D] (contiguous load)
    return np.ascontiguousarray(
        np.asarray(w, np.float32).reshape(CT, 128, D).transpose(1, 0, 2)
    ).astype(NP_BF16).reshape(128, CT * D)


def _warr_dt(w):
    # [d_in, d_out] -> [p, dt, ct, c] flattened (dt-major: chunk 0 loads first)
    return np.ascontiguousarray(
        np.asarray(w, np.float32).reshape(CT, 128, CT, 128).transpose(1, 2, 0, 3)
    ).astype(NP_BF16).reshape(128, CT * D)


def make_in_maps(x, W_q, W_k, W_v, W_o):
    wq = _warr_dt(np.asarray(W_q, np.float32) * 0.125)
    wk = _warr_dt(W_k)
    wv = _warr_dt(W_v)
    wo = _warr(W_o)
    x = np.asarray(x, np.float32)
    in_maps = []
    for r in range(R):
        shard = x[:, r * SQ : (r + 1) * SQ, :].reshape(ROWS, D)  # batch-major rows
        xT_r = np.ascontiguousarray(
            shard.T.reshape(CT, 128, ROWS).transpose(1, 0, 2)
        ).astype(NP_BF16).reshape(128, CT * ROWS)
        in_maps.append({"xT": xT_r, "wq": wq, "wk": wk, "wv": wv, "wo": wo})
    return in_maps


def assemble_out(results):
    full = np.zeros((B, S, D), np.float32)
    for r in range(R):
        o = np.asarray(results[r]["out"], np.float32)
        for b in range(B):
            full[b, r * SQ : (r + 1) * SQ, :] = o[b * SQ : (b + 1) * SQ, :]
    return full


def run(x, W_q, W_k, W_v, W_o, trace=False):
    nc = _get_nc()
    in_maps = make_in_maps(x, W_q, W_k, W_v, W_o)
    res = run_bass_kernel_spmd(nc, in_maps, core_ids=list(range(R)), trace=trace)
    return assemble_out(res.results), res


def kernel(x, W_q, W_k, W_v, W_o):
    out, _ = run(x, W_q, W_k, W_v, W_o)
    return out
